# revision 15
# baseline (speedup 1.0000x reference)
"""Trainium2 Bass kernel for the DivTree per-agent MoE MLP problem.

Math (see reference): for each agent c of 64,
  x2[:, c, :]      = relu(x[:, c, :] @ W1[r[c]][:H] + bias_A[c]) @ W2[r[c]] + b2[r[c]]
  confact[:, c, :] = relu(x[:, c, :] @ W1[r'[c]][:H] + bias_B[c]) @ W2[r'[c]] + b2[r'[c]]
where r = routing, r'[c] = routing[pinv[c]] with pinv the inverse of perm_index,
and the one-hot agent-id concat folds into the bias:
  bias_A[c] = b1[r[c]] + W1[r[c]][H + c],  bias_B[c] = b1[r'[c]] + W1[r'[c]][H + pinv[c]].

Work dedup: when r'[c] == r[c] both passes share the layer-1 MATMUL
x_c @ W1[e] — only the folded one-hot bias differs — so the PSUM
accumulation is computed once and consumed by two bias+relu epilogues
("S slots"). Columns with distinct experts run two full passes off one
x^T tile ("P slots"). Work is split at unit granularity (1024 of the
2048 batch): with the staged routing/perm, 52 P-columns and 12 S-columns
x 2 units = 104 P + 24 S unit-slots = exactly 13 P + 3 S per core.

Device layout per core (NS = n2+n3 slots, NW = 2*n2+n3 L1 weight sets,
NJ = 2*(n2+n3) L2 jobs):
  xs  [128, NS, KC, U]   x^T slot tiles
  w1  [128, NW, KC, H]   layer-1 lhsT (k-part, k-chunk, m)
  w2  [128, NJ, KC, 128] layer-2 lhsT (O=32 replicated x4 in m)
  b1  [128, NJ, MT]      per-partition layer-1 bias (one-hot folded in)
  b2  [32, NJ]           per-partition layer-2 bias
  out [NJ, O, U]         transposed per-job output

Matmuls run in float32r (measured on HW: full-rate 1 col/cycle, and
FASTER end-to-end than bf16 on this workload despite 2x the DMA bytes);
fp32 PSUM accumulation and fp32 biases throughout, rel err ~2e-4.
Layer-2 outputs of 4 consecutive jobs are packed into one [128, 1024]
SBUF tile so the out-DMA runs at full 128-partition width (4x fewer,
4x wider writes). A 2-job software skew keeps independent layer-1
matmuls on the PE while each job's PSUM epilogues drain.

Measured (in-NEFF repeat-delta, device-resident inputs): ~258-282 us
per iteration depending on session conditions; PE-busy floor for this
dataflow is ~253 us/core (606k PE cycles at 2.4 GHz, 93-97% occupancy).

Prefetch depth (xbufs=4, wbufs=6 vs the original 3/4) is the main lever
found on top of that: it hides x/W1 DMA latency jitter and measured
~240-259 us vs ~271-322 us for the shallower original, winning every
within-run A/B across five sessions. Alternatives evaluated and
rejected on-hardware: fp8e4-DoubleRow (true 2x per-MAC rate confirmed
by microbenchmark, but raw fp8 L1 gives rel_total 3.9e-2 > the 2e-2
gate, and hi/lo-pair compensation exactly cancels the speedup);
transposed L2 with h stationary (75.9 ns/instr — LDW not hidden at
N=32, worse than the replicated-m layout); paired W1/W2 loads for
(c,0),(c,1) slot pairs (correct, -17MB/core DMA, but 261.6 vs 240.4 us
in-session — shared-tile lifetimes cost more than the DMA saving);
bf16 matmuls (measured 9% slower streaming than f32r); int8 (no such
matmul dtype in Bass). 606k PE cycles/core is the floor for this
algorithm at >=bf16 accuracy; the kernel sits at 95%+ of it.
"""

import ml_dtypes
import numpy as np

import concourse.bass as bass
import concourse.mybir as mybir
from concourse import bacc
from concourse.tile import TileContext
from concourse.bass_utils import run_bass_kernel_spmd

F32 = mybir.dt.float32
F32R = mybir.dt.float32r
BF16 = mybir.dt.bfloat16
AF = mybir.ActivationFunctionType
ALU = mybir.AluOpType

B, A, H, O = 2048, 64, 512, 32
NCORES = 8
NU = 2                    # batch units per full pass
U = B // NU               # 1024
KC = H // 128             # 4 k-chunks
MT = H // 128             # 4 layer-1 output tiles

DT_L1 = F32R              # layer-1 matmul input dtype (x, W1)
DT_L2 = F32R              # layer-2 matmul input dtype (h, W2)


def _npdt(dt):
    return np.float32 if dt == F32R else ml_dtypes.bfloat16


_CACHED = {}


def _build_nc(repeat=1, n2=13, n3=3, skew=2, xbufs=4, wbufs=6, hbufs=4,
              half_ps=False, ps1_bufs=None, ob_bufs=3, pairing=None,
              w2bufs=None):
    """n2 P-slots (2 independent passes) + n3 S-slots (shared L1 matmul).

    pairing=(np2, ns2p, np3, ns3): per-core slots are [np2 P-pairs,
    ns2p P-singles, np3 S-pairs, ns3 S-singles]; a pair = the (c,0),(c,1)
    unit-slots of one column, sharing W1/W2 dram entries (loaded once).
    """
    ns = n2 + n3
    nw = 2 * n2 + n3
    nj = 2 * ns
    if pairing is not None:
        np2p, ns2p, np3p, ns3p = pairing
        nw = 2 * np2p + 2 * ns2p + np3p + ns3p
    nc = bacc.Bacc("TRN2", target_bir_lowering=False, debug=False,
                   num_devices=NCORES)
    xs = nc.dram_tensor("xs", [128, ns, KC, U], DT_L1, kind="ExternalInput")
    w1 = nc.dram_tensor("w1", [128, nw, KC, H], DT_L1, kind="ExternalInput")
    w2 = nc.dram_tensor("w2", [128, nj if pairing is None else nw, KC, 128],
                        DT_L2, kind="ExternalInput")
    b1 = nc.dram_tensor("b1", [128, nj, MT], F32, kind="ExternalInput")
    b2 = nc.dram_tensor("b2", [32, nj], F32, kind="ExternalInput")
    out = nc.dram_tensor("out", [nj * O, U], F32, kind="ExternalOutput")

    with TileContext(nc) as tc:
        with (
            tc.tile_pool(name="weights", bufs=1) as wpool,
            tc.tile_pool(name="w1s", bufs=wbufs) as w1pool,
            tc.tile_pool(name="w2s", bufs=(w2bufs or (6 if pairing else 4))) as w2pool,
            tc.tile_pool(name="xT", bufs=xbufs) as xtpool,
            tc.tile_pool(name="hT", bufs=hbufs) as hpool,
            tc.tile_pool(name="ob", bufs=ob_bufs) as opool,
            tc.tile_pool(name="ps1", bufs=(ps1_bufs or (6 if half_ps else 3)),
                         space="PSUM") as ps1_pool,
            tc.tile_pool(name="ps2", bufs=2, space="PSUM") as ps2_pool,
        ):
            b1t = wpool.tile([128, nj, MT], F32)
            nc.sync.dma_start(b1t[:], b1[:])
            b2t = wpool.tile([32, nj], F32)
            nc.sync.dma_start(b2t[:], b2[:])

            # epilogue ops alternate DVE/Act via a running counter
            ecnt = [0]

            def epi1(dst, src, bias_ap):
                if ecnt[0] % 2 == 0:
                    nc.vector.tensor_scalar(dst, src, bias_ap, 0.0,
                                            ALU.add, ALU.max)
                else:
                    nc.scalar.activation(dst, src, AF.Relu, bias=bias_ap)
                ecnt[0] += 1

            def epi2(dst, src, bias_ap):
                if ecnt[0] % 2 == 0:
                    nc.scalar.activation(dst, src, AF.Identity, bias=bias_ap)
                else:
                    nc.vector.tensor_scalar_add(dst, src, bias_ap)
                ecnt[0] += 1

            def l1_matmuls(ps1, xT, w1t, mt):
                for kc in range(KC):
                    lhsT = w1t[:, kc, mt * 128:(mt + 1) * 128]
                    for half in range(2):
                        nc.tensor.matmul(
                            ps1[:, half * 512:(half + 1) * 512],
                            lhsT,
                            xT[:, kc, half * 512:(half + 1) * 512],
                            start=(kc == 0), stop=(kc == KC - 1),
                        )

            def l1_matmuls_half(ps1, xT, w1t, mt, half):
                for kc in range(KC):
                    nc.tensor.matmul(
                        ps1[:],
                        w1t[:, kc, mt * 128:(mt + 1) * 128],
                        xT[:, kc, half * 512:(half + 1) * 512],
                        start=(kc == 0), stop=(kc == KC - 1),
                    )

            def layer1(xT, w1t, j):
                hT = hpool.tile([128, KC, U], DT_L2, tag="h")
                for mt in range(MT):
                    if half_ps:
                        for half in range(2):
                            ps1 = ps1_pool.tile([128, 512], F32)
                            l1_matmuls_half(ps1, xT, w1t, mt, half)
                            epi1(hT[:, mt, half * 512:(half + 1) * 512],
                                 ps1[:], b1t[:, j, mt:mt + 1])
                    else:
                        ps1 = ps1_pool.tile([128, U], F32)
                        l1_matmuls(ps1, xT, w1t, mt)
                        epi1(hT[:, mt, :], ps1[:], b1t[:, j, mt:mt + 1])
                return hT

            def layer1_shared(xT, w1t, jA, jB):
                hA = hpool.tile([128, KC, U], DT_L2, tag="h")
                hB = hpool.tile([128, KC, U], DT_L2, tag="h")
                for mt in range(MT):
                    if half_ps:
                        for half in range(2):
                            ps1 = ps1_pool.tile([128, 512], F32)
                            l1_matmuls_half(ps1, xT, w1t, mt, half)
                            sl = slice(half * 512, (half + 1) * 512)
                            epi1(hA[:, mt, sl], ps1[:], b1t[:, jA, mt:mt + 1])
                            epi1(hB[:, mt, sl], ps1[:], b1t[:, jB, mt:mt + 1])
                    else:
                        ps1 = ps1_pool.tile([128, U], F32)
                        l1_matmuls(ps1, xT, w1t, mt)
                        epi1(hA[:, mt, :], ps1[:], b1t[:, jA, mt:mt + 1])
                        epi1(hB[:, mt, :], ps1[:], b1t[:, jB, mt:mt + 1])
                return hA, hB

            def layer2(hT, j, ob, w2t):
                # ob is a [32, U] slice of a 4-job group tile; the group
                # DMA is issued by the caller once all 4 rows are written
                for half in range(2):
                    ps2 = ps2_pool.tile([128, 512], F32)
                    for kc in range(KC):
                        nc.tensor.matmul(
                            ps2[:],
                            w2t[:, kc, :],
                            hT[:, kc, half * 512:(half + 1) * 512],
                            start=(kc == 0), stop=(kc == KC - 1),
                        )
                    epi2(ob[:, half * 512:(half + 1) * 512], ps2[:32, :],
                         b2t[:, j:j + 1])

            # Software-skew state persists across repeat iterations so the
            # pipeline never drains at a body boundary (the tail layer-2s
            # of body i overlap the first layer-1s of body i+1).
            pending = []
            group = [None, 0]         # current 4-job output tile, fill count

            def body(drain):

                def run_one(hT, j, ob, w2t):
                    layer2(hT, j, ob, w2t)
                    group[1] += 1
                    if group[1] == 4 or j == nj - 1:
                        g = group[1]
                        nc.sync.dma_start(
                            out[(j - g + 1) * O:(j + 1) * O, :],
                            group[0][:32 * g, :],
                        )
                        group[0], group[1] = None, 0

                def queue(hT, j, w2t):
                    pending.append((hT, j, w2t))
                    if len(pending) > skew:
                        hT0, j0, w2t0 = pending.pop(0)
                        if group[0] is None:
                            group[0] = opool.tile([128, U], F32, name="ob4",
                                                  tag="ob4")
                        ob = group[0][32 * group[1]:32 * (group[1] + 1), :]
                        run_one(hT0, j0, ob, w2t0)

                def load_w1(idx):
                    w1t = w1pool.tile([128, KC, H], DT_L1)
                    nc.sync.dma_start(w1t[:], w1[:, idx, :, :])
                    return w1t

                def load_w2(idx):
                    w2t = w2pool.tile([128, KC, 128], DT_L2)
                    nc.sync.dma_start(w2t[:], w2[:, idx, :, :])
                    return w2t

                def load_x(t):
                    xT = xtpool.tile([128, KC, U], DT_L1)
                    nc.sync.dma_start(xT[:], xs[:, t, :, :])
                    return xT

                if pairing is None:
                    for t in range(ns):
                        xT = load_x(t)
                        if t < n2:
                            for k in range(2):
                                w1t = load_w1(2 * t + k)
                                queue(layer1(xT, w1t, 2 * t + k), 2 * t + k,
                                      load_w2(2 * t + k))
                        else:
                            w1t = load_w1(n2 + t)
                            hA, hB = layer1_shared(xT, w1t, 2 * t, 2 * t + 1)
                            queue(hA, 2 * t, load_w2(2 * t))
                            queue(hB, 2 * t + 1, load_w2(2 * t + 1))
                else:
                    # per-core layout: [np2p P-pairs | ns2p P-singles |
                    #                   np3p S-pairs | ns3p S-singles]
                    def p_slot(t, w1a, w1b, w2a, w2b):
                        xT = load_x(t)
                        queue(layer1(xT, w1a, 2 * t), 2 * t, w2a)
                        queue(layer1(xT, w1b, 2 * t + 1), 2 * t + 1, w2b)

                    def s_slot(t, w1t, w2t):
                        xT = load_x(t)
                        hA, hB = layer1_shared(xT, w1t, 2 * t, 2 * t + 1)
                        queue(hA, 2 * t, w2t)
                        queue(hB, 2 * t + 1, w2t)

                    for i in range(np2p):         # P pairs
                        w1a, w1b = load_w1(2 * i), load_w1(2 * i + 1)
                        w2a, w2b = load_w2(2 * i), load_w2(2 * i + 1)
                        p_slot(2 * i, w1a, w1b, w2a, w2b)
                        p_slot(2 * i + 1, w1a, w1b, w2a, w2b)
                    for sidx in range(ns2p):      # P singles
                        t = 2 * np2p + sidx
                        e0 = 2 * np2p + 2 * sidx
                        p_slot(t, load_w1(e0), load_w1(e0 + 1),
                               load_w2(e0), load_w2(e0 + 1))
                    pbase = 2 * np2p + 2 * ns2p
                    for jdx in range(np3p):       # S pairs
                        t0 = n2 + 2 * jdx
                        w1t = load_w1(pbase + jdx)
                        w2t = load_w2(pbase + jdx)
                        s_slot(t0, w1t, w2t)
                        s_slot(t0 + 1, w1t, w2t)
                    for sidx in range(ns3p):      # S singles
                        t = n2 + 2 * np3p + sidx
                        s_slot(t, load_w1(pbase + np3p + sidx),
                               load_w2(pbase + np3p + sidx))
                if drain:
                    while pending:
                        hT0, j0, w2t0 = pending.pop(0)
                        if group[0] is None:
                            group[0] = opool.tile([128, U], F32, name="ob4",
                                                  tag="ob4")
                        ob = group[0][32 * group[1]:32 * (group[1] + 1), :]
                        run_one(hT0, j0, ob, w2t0)

            for r in range(repeat):
                body(drain=(r == repeat - 1))
    nc.compile()
    return nc


FORCE_DENSE = False
# Pairing (shared W1/W2 loads for (c,0),(c,1) slot pairs) measured 261.6us
# vs 240.4us unpaired in the same session: the shared-tile lifetimes cost
# more pipeline freedom than the 17MB/core DMA saving buys. Keep it off.
FORCE_UNPAIRED = True


def _plan(routing, perm_index):
    """Slot plan from the actual routing+perm.

    Returns (n2, n3, slot_map): slot_map[core] = list of slots, each slot
    (c, u, kind) with kind 'P' (two experts) or 'S' (shared expert).
    """
    routing = np.asarray(routing).astype(np.int64)
    perm = np.asarray(perm_index).astype(np.int64)
    pinv = np.empty(A, dtype=np.int64)
    pinv[perm] = np.arange(A)
    eA, eB = routing, routing[pinv]

    if FORCE_DENSE:
        p_slots = [(c, u) for c in range(A) for u in range(NU)]
        s_slots = []
    else:
        p_slots = [(c, u) for c in range(A) if eA[c] != eB[c]
                   for u in range(NU)]
        s_slots = [(c, u) for c in range(A) if eA[c] == eB[c]
                   for u in range(NU)]
    # pad to a uniform per-core count (padded slots recompute an existing
    # slot; their outputs are ignored at assembly)
    while p_slots and len(p_slots) % NCORES:
        p_slots.append(p_slots[0])
    while s_slots and len(s_slots) % NCORES:
        s_slots.append(s_slots[0])
    n2 = len(p_slots) // NCORES
    n3 = len(s_slots) // NCORES

    def reorder(slots):
        """Group a core's slots into aligned (c,0),(c,1) pairs + singles.

        Returns (ordered_slots, n_pairs, n_singles) with pairs first.
        """
        from collections import OrderedDict
        byc = OrderedDict()
        rest = []
        for (c, u) in slots:
            byc.setdefault(c, []).append((c, u))
        pairs, singles = [], []
        for c, lst in byc.items():
            if len(lst) == 2 and {u for _, u in lst} == {0, 1}:
                pairs.extend(sorted(lst, key=lambda s: s[1]))
            else:
                singles.extend(lst)
        return pairs + singles, len(pairs) // 2, len(singles)

    slot_map = []
    structs = set()
    for core in range(NCORES):
        p, np2, ns2p = reorder([p_slots[core * n2 + t] for t in range(n2)])
        s, np3, ns3 = reorder([s_slots[core * n3 + t] for t in range(n3)])
        structs.add((np2, ns2p, np3, ns3))
        slots = [ps + ("P",) for ps in p] + [ss + ("S",) for ss in s]
        slot_map.append(slots)
    pairing = structs.pop() if len(structs) == 1 else None
    if pairing is not None and pairing[0] == 0 and pairing[2] == 0:
        pairing = None          # no pairs anywhere: use the plain path
    if FORCE_UNPAIRED:
        pairing = None
    return n2, n3, slot_map, pairing


_LAST_PLAN = {}


def _w1_entries(slot_map_core, n2, pairing, eA, eB):
    """Ordered list of (expert, onehot-agent-or-None) w1/w2 dram entries.

    Mirrors the _build_nc paired traversal exactly. Returns list of
    experts; entry k of w1/w2 holds that expert's weights.
    """
    np2, ns2p, np3, ns3 = pairing
    ents = []
    for i in range(np2):            # P pairs: 2 entries each (eA, eB)
        c = slot_map_core[2 * i][0]
        ents += [eA[c], eB[c]]
    for s in range(ns2p):           # P singles: 2 entries each
        c = slot_map_core[2 * np2 + s][0]
        ents += [eA[c], eB[c]]
    for j in range(np3):            # S pairs: 1 entry each
        c = slot_map_core[n2 + 2 * j][0]
        ents.append(eA[c])
    for s in range(ns3):            # S singles: 1 entry each
        c = slot_map_core[n2 + 2 * np3 + s][0]
        ents.append(eA[c])
    return ents


def _host_prep(x_in, W1, b1, W2, b2, routing, perm_index):
    n2, n3, slot_map, pairing = _plan(routing, perm_index)
    ns, nw, nj = n2 + n3, 2 * n2 + n3, 2 * (n2 + n3)

    routing = np.asarray(routing).astype(np.int64)
    perm = np.asarray(perm_index).astype(np.int64)
    pinv = np.empty(A, dtype=np.int64)
    pinv[perm] = np.arange(A)
    eA, eB = routing, routing[pinv]

    x_in = np.asarray(x_in, dtype=np.float32)
    W1 = np.asarray(W1, dtype=np.float32)
    b1 = np.asarray(b1, dtype=np.float32)
    W2 = np.asarray(W2, dtype=np.float32)
    b2 = np.asarray(b2, dtype=np.float32)
    np1, np2 = _npdt(DT_L1), _npdt(DT_L2)

    # x^T: [hi, c, u, kc, f]
    xT_all = np.ascontiguousarray(
        x_in.reshape(NU, U, A, KC, 128).transpose(4, 2, 0, 3, 1))
    used = sorted(set(routing.tolist()))
    w1e = {e: np.ascontiguousarray(
        W1[e, :H, :].reshape(KC, 128, H).transpose(1, 0, 2)).astype(np1)
        for e in used}                       # [128, KC, H]
    w2e = {e: np.ascontiguousarray(np.tile(
        W2[e].reshape(KC, 128, O), (1, 1, 4)).transpose(1, 0, 2)
        ).astype(np2)                        # [128, KC, 128]
        for e in used}

    def bias1(e, oh):
        return (b1[e] + W1[e, H + oh, :]).reshape(MT, 128).T

    nwe = nw if pairing is None else (
        2 * pairing[0] + 2 * pairing[1] + pairing[2] + pairing[3])

    in_maps = []
    for core in range(NCORES):
        xs_c = np.empty((128, ns, KC, U), dtype=np1)
        w2_c = np.empty((128, nj if pairing is None else nwe, KC, 128),
                        dtype=np2)
        b1_c = np.empty((128, nj, MT), dtype=np.float32)
        b2_c = np.empty((32, nj), dtype=np.float32)
        if pairing is None:
            w1_c = np.empty((128, nw, KC, H), dtype=np1)
        else:
            ents = _w1_entries(slot_map[core], n2, pairing, eA, eB)
            assert len(ents) == nwe
            w1_c = np.stack([w1e[e] for e in ents], axis=1)
            for k, e in enumerate(ents):
                w2_c[:, k] = w2e[e]
        for t, (c, u, kind) in enumerate(slot_map[core]):
            xs_c[:, t] = xT_all[:, c, u]
            jA, jB = 2 * t, 2 * t + 1
            if pairing is None:
                if kind == "P":
                    w1_c[:, 2 * t] = w1e[eA[c]]
                    w1_c[:, 2 * t + 1] = w1e[eB[c]]
                else:
                    w1_c[:, n2 + t] = w1e[eA[c]]
                w2_c[:, jA] = w2e[eA[c]]
                w2_c[:, jB] = w2e[eB[c]]
            b1_c[:, jA] = bias1(eA[c], c)
            b1_c[:, jB] = bias1(eB[c], int(pinv[c]))
            b2_c[:, jA] = b2[eA[c]]
            b2_c[:, jB] = b2[eB[c]]
        in_maps.append({"xs": xs_c, "w1": w1_c, "w2": w2_c,
                        "b1": b1_c, "b2": b2_c})
    _LAST_PLAN.clear()
    _LAST_PLAN.update({"n2": n2, "n3": n3, "pairing": pairing})
    return in_maps


def _plan_kwargs(in_maps):
    return dict(_LAST_PLAN)


def kernel(x_in, W1, b1, W2, b2, routing, perm_index):
    n2, n3, slot_map, pairing = _plan(routing, perm_index)
    key = (n2, n3, pairing)
    if key not in _CACHED:
        _CACHED[key] = _build_nc(n2=n2, n3=n3, pairing=pairing)
    nc = _CACHED[key]

    in_maps = _host_prep(x_in, W1, b1, W2, b2, routing, perm_index)
    res = run_bass_kernel_spmd(nc, in_maps, list(range(NCORES)))

    x2 = np.empty((B, A, O), dtype=np.float32)
    confact = np.empty((B, A, O), dtype=np.float32)
    seen = set()
    nj = 2 * (n2 + n3)
    for core in range(NCORES):
        o = res.results[core]["out"].reshape(nj, O, U)
        for t, (c, u, kind) in enumerate(slot_map[core]):
            for s, j in ((0, 2 * t), (1, 2 * t + 1)):
                if (c, s, u) in seen:
                    continue                      # padded duplicate slot
                seen.add((c, s, u))
                dst = x2 if s == 0 else confact
                dst[u * U:(u + 1) * U, c, :] = o[j].T
    return x2, confact

